# revision 39
# baseline (speedup 1.0000x reference)
"""Trainium2 Bass kernel for an AttentionBlock (GroupNorm -> 1-head attention -> proj -> residual).

Problem: hidden_states (4, 512, 64, 64) fp32; GroupNorm(32 groups) then
single-head attention over S=4096 tokens with head_dim=C=512, output
projection, residual add.

Sharding: 8 cores = 4 batch elements x 2 query-halves. Each core:
 - receives the full [512, 4096] (channels x spatial) slab for its batch
   element, spatially rotated so that *its* 2048 queries are columns 0:2048
   (attention is permutation-invariant over keys, so every core can run the
   identical SPMD program);
 - computes GroupNorm + K/V for all 4096 tokens (redundant x2, cheap) and
   Q only for its half;
 - computes scores^T (keys-on-partition layout), exp, attn @ V, out-proj,
   residual -- no on-chip transposes anywhere.

Numerics: fp16 matmul operands with fp32 PSUM accumulation; softmax without
max-subtraction (scores ~ N(0,1)) but with a constant exp-bias of -4 to keep
unnormalized sums in fp16 range; normalization deferred past the output
projection ((P@V)@Wo / den == (P/den @ V)@Wo). Measured end-to-end absmax
relative error vs fp32 reference: 4.3e-5.

Measured on 8 axon TRN2 cores: ~381us HW exec (~455us when the chip sits in
its throttled P0 clock state); TensorE occupancy ~90%.
"""

from contextlib import ExitStack

import numpy as np

import concourse.bacc as bacc
import concourse.bass as bass
import concourse.tile as tile
from concourse import mybir
from concourse.bass_utils import run_bass_kernel_spmd

F32 = mybir.dt.float32
F16 = mybir.dt.float16

B = 4
C = 512
S = 4096  # 64*64 tokens
SH = S // 2  # tokens per core (query half)
GROUPS = 32
GSIZE = C // GROUPS  # 16 channels per group
EPS = 1e-6
CT = C // 128  # 4 channel tiles
SCALE = 1.0 / np.sqrt(np.float32(C))
EXPBIAS = -4.0  # constant max-substitute inside exp; cancels in normalization

N_CORES = 8


def _build_kernel(ctx: ExitStack, tc: tile.TileContext, d):
    nc = tc.nc
    mult = mybir.AluOpType.mult
    add = mybir.AluOpType.add
    subtract = mybir.AluOpType.subtract
    Act = mybir.ActivationFunctionType

    cst = ctx.enter_context(tc.tile_pool(name="cst", bufs=1))
    xin = ctx.enter_context(tc.tile_pool(name="xin", bufs=3))
    gnp = ctx.enter_context(tc.tile_pool(name="gnp", bufs=4))
    big = ctx.enter_context(tc.tile_pool(name="big", bufs=1))
    expp = ctx.enter_context(tc.tile_pool(name="expp", bufs=3))
    smal = ctx.enter_context(tc.tile_pool(name="smal", bufs=2))
    resp = ctx.enter_context(tc.tile_pool(name="resp", bufs=2))
    dramp = ctx.enter_context(tc.tile_pool(name="dramp", bufs=2, space="DRAM"))
    finp = ctx.enter_context(tc.tile_pool(name="finp", bufs=2))

    x_d = d["x"]  # fp16 copy of the input slab: GN stats + matmul path
    # gmat/gn-affine first (tiny): they gate the GroupNorm matmul + normalize
    gmat_raw = cst.tile([128, 128], F32, tag="gmat_raw")
    nc.sync.dma_start(out=gmat_raw[:], in_=d["gmat"][:])
    gw_sb = cst.tile([128, CT], F32, tag="gw")
    nc.sync.dma_start(out=gw_sb[:], in_=d["gw2"][:])
    gb_sb = cst.tile([128, CT], F32, tag="gb")
    nc.sync.dma_start(out=gb_sb[:], in_=d["gb2"][:])
    # x next: its DMA latency gates GroupNorm and everything after. Two
    # sub-DMAs per channel tile so bn_stats can start on the first half early;
    # each tile gets its own slot so all four transfers issue immediately.
    x_tiles = []
    for t in range(CT):
        x_t = xin.tile([128, S], F16, tag=f"xt{t}", name=f"xt{t}", bufs=1)
        for h in range(4):
            nc.sync.dma_start(
                out=x_t[:, h * (S // 4) : (h + 1) * (S // 4)],
                in_=x_d[t * 128 : (t + 1) * 128, h * (S // 4) : (h + 1) * (S // 4)],
            )
        x_tiles.append(x_t)

    # ---- constants / weights to SBUF (gpsimd DMA queue; overlaps x).
    # Order = first-use order: K/Q/V weights gate the projections (~15us),
    # biases gate the PSUM->SBUF copies a bit later, wo3/bo much later.
    wq3 = cst.tile([128, CT, C], F16, tag="wq3")
    wk3 = cst.tile([128, CT, C], F16, tag="wk3")
    wv3 = cst.tile([128, CT, C], F16, tag="wv3")
    wo3 = cst.tile([128, CT, C], F16, tag="wo3")
    for w_sb, w_d in ((wk3, d["wkt"]), (wq3, d["wqt"]), (wv3, d["wvt"])):
        nc.gpsimd.dma_start(out=w_sb[:], in_=w_d.rearrange("(t p) o -> p t o", p=128))
    bq_sb = cst.tile([128, CT], F32, tag="bq")
    bk_sb = cst.tile([128, CT], F32, tag="bk")
    bo_sb = cst.tile([128, CT], F32, tag="bo")
    for t_sb, t_d in ((bk_sb, d["bk2"]), (bq_sb, d["bq2"]), (bo_sb, d["bo2"])):
        nc.gpsimd.dma_start(out=t_sb[:], in_=t_d[:])
    bvb_sb = cst.tile([128, C], F32, tag="bvb")
    nc.gpsimd.dma_start(out=bvb_sb[:], in_=d["bvb"][:])
    nc.gpsimd.dma_start(out=wo3[:], in_=d["wot"].rearrange("(t p) o -> p t o", p=128))
    # staging copy: the first PE matmul then depends only on the DVE
    # semaphore (S3_LW allows a single wait)
    gmat_sb = cst.tile([128, 128], F32, tag="gmat")
    nc.vector.tensor_copy(out=gmat_sb[:], in_=gmat_raw[:])
    ones_sb = cst.tile([128, 128], F32, tag="ones")
    nc.vector.memset(ones_sb[:], 1.0)
    eps_t = cst.tile([128, 1], F32, tag="epsc")
    nc.vector.memset(eps_t[:], float(EPS))
    expb_t = cst.tile([128, 1], F32, tag="expb")
    nc.vector.memset(expb_t[:], float(EXPBIAS))

    # proj-phase PSUM pool: 6 banks; scoped so its banks are released to the
    # attention pools afterwards
    proj_ctx = ExitStack()
    pjsum = proj_ctx.enter_context(tc.tile_pool(name="pjsum", bufs=6, space="PSUM"))

    # PE warmup: keep TensorE busy during the initial x DMA so HAM reaches
    # K=8/8 before real matmuls; fp32 ones matmuls, one PSUM bank, serial.
    wu = pjsum.tile([128, 128], F32, tag="wu", bufs=1)
    for _ in range(30):
        nc.tensor.matmul(wu[:], lhsT=ones_sb[:], rhs=ones_sb[:], start=True, stop=True)
    # ---- GroupNorm: per channel-tile stats -> group stats -> normalize ----
    xg3 = big.tile([128, CT, S], F16, tag="xg3")  # normalized input, [c, s]
    for t in range(CT):
        x_t = x_tiles[t]
        stats = gnp.tile([128, 8, 6], F32, tag="stats")
        xr = x_t.rearrange("p (n f) -> p n f", f=512)
        for i in range(8):
            nc.vector.bn_stats(out=stats[:, i, :], in_=xr[:, i, :])
        mv = gnp.tile([128, 2], F32, tag="mv")
        nc.vector.bn_aggr(out=mv[:], in_=stats[:])

        # group-average mean/var and mean^2 across the 16 partitions of each
        # group via PE matmuls; each rhs has a single producer instruction
        # (matmul LDWEIGHTS lowering only supports a couple of sem waits).
        mv3 = gnp.tile([128, 3], F32, tag="mv3")
        nc.vector.tensor_copy(out=mv3[:, 0:2], in_=mv[:])
        nc.vector.tensor_tensor(out=mv3[:, 2:3], in0=mv[:, 0:1], in1=mv[:, 0:1], op=mult)
        ps_g = pjsum.tile([128, 3], F32, tag="pj")
        nc.tensor.matmul(ps_g[:], lhsT=gmat_sb[:], rhs=mv3[:], start=True, stop=True)

        # gstat = [mean_g, avg var_p, avg mean_p^2]
        gstat = gnp.tile([128, 3], F32, tag="gstat")
        nc.vector.tensor_copy(out=gstat[:], in_=ps_g[:])
        # var_g = avg(var_p) + avg(mean_p^2) - mean_g^2
        varg = gnp.tile([128, 1], F32, tag="varg")
        nc.vector.tensor_tensor(out=varg[:], in0=gstat[:, 0:1], in1=gstat[:, 0:1], op=mult)
        nc.vector.tensor_tensor(out=varg[:], in0=gstat[:, 2:3], in1=varg[:], op=subtract)
        nc.vector.tensor_tensor(out=varg[:], in0=gstat[:, 1:2], in1=varg[:], op=add)
        stdt = gnp.tile([128, 1], F32, tag="stdt")
        nc.scalar.activation(out=stdt[:], in_=varg[:], func=Act.Sqrt, bias=eps_t[:])
        rstd = gnp.tile([128, 1], F32, tag="rstd")
        nc.vector.reciprocal(out=rstd[:], in_=stdt[:])

        scl = gnp.tile([128, 1], F32, tag="scl")
        nc.vector.tensor_tensor(out=scl[:], in0=rstd[:], in1=gw_sb[:, t : t + 1], op=mult)
        sft = gnp.tile([128, 1], F32, tag="sft")
        nc.vector.tensor_tensor(out=sft[:], in0=gstat[:, 0:1], in1=scl[:], op=mult)
        nc.vector.tensor_tensor(out=sft[:], in0=gb_sb[:, t : t + 1], in1=sft[:], op=subtract)

        # alternate the normalize between ACT and DVE so consecutive channel
        # tiles normalize in parallel on the two engines
        if t % 2 == 0:
            nc.scalar.activation(
                out=xg3[:, t, :],
                in_=x_t[:],
                func=Act.Identity,
                bias=sft[:],
                scale=scl[:],
            )
        else:
            nc.vector.tensor_scalar(
                out=xg3[:, t, :],
                in0=x_t[:],
                scalar1=scl[:],
                scalar2=sft[:],
                op0=mult,
                op1=add,
            )

    # ---- projections ----
    kt3 = big.tile([128, CT, S], F16, tag="kt3")  # k^T [c, j]
    for ot in range(CT):
        for jc in range(S // 512):
            ps = pjsum.tile([128, 512], F32, tag="pj")
            for t in range(CT):
                nc.tensor.matmul(
                    ps[:],
                    lhsT=wk3[:, t, ot * 128 : (ot + 1) * 128],
                    rhs=xg3[:, t, jc * 512 : (jc + 1) * 512],
                    start=(t == 0),
                    stop=(t == CT - 1),
                )
            nc.scalar.activation(
                out=kt3[:, ot, jc * 512 : (jc + 1) * 512],
                in_=ps[:],
                func=Act.Identity,
                bias=bk_sb[:, ot : ot + 1],
            )

    qt3 = big.tile([128, CT, SH], F16, tag="qt3")  # q^T [c, i]
    for ot in range(CT):
        for ic in range(SH // 512):
            ps = pjsum.tile([128, 512], F32, tag="pj")
            for t in range(CT):
                nc.tensor.matmul(
                    ps[:],
                    lhsT=wq3[:, t, ot * 128 : (ot + 1) * 128],
                    rhs=xg3[:, t, ic * 512 : (ic + 1) * 512],
                    start=(t == 0),
                    stop=(t == CT - 1),
                )
            nc.scalar.activation(
                out=qt3[:, ot, ic * 512 : (ic + 1) * 512],
                in_=ps[:],
                func=Act.Identity,
                bias=bq_sb[:, ot : ot + 1],
            )

    v3 = big.tile([128, S // 128, C], F16, tag="v3")  # v natural [j, o]
    for jb in range(S // 128):
        ps = pjsum.tile([128, 512], F32, tag="pj")
        for t in range(CT):
            nc.tensor.matmul(
                ps[:],
                lhsT=xg3[:, t, jb * 128 : (jb + 1) * 128],
                rhs=wv3[:, t, :],
                start=(t == 0),
                stop=(t == CT - 1),
            )
        nc.vector.tensor_add(out=v3[:, jb, :], in0=ps[:], in1=bvb_sb[:])

    # release the 6 proj banks, then open the attention PSUM pools:
    # ps(2) + av0..3(1 each) + pp(2) = 8 banks
    proj_ctx.close()
    ppsum = ctx.enter_context(tc.tile_pool(name="ppsum", bufs=2, space="PSUM"))
    apsum = ctx.enter_context(tc.tile_pool(name="apsum", bufs=1, space="PSUM"))

    # ---- attention + output projection, per 512-query chunk ----
    # The per-chunk epilogue (denominator, attn-out copies, output projection,
    # residual) is deferred into the next chunk's j-loop so its PE work and
    # PSUM->SBUF copies overlap the next chunk's score matmuls.
    NJB = S // 128  # 32 key blocks

    def make_finisher(ic, av, sums):
        isl = slice(ic * 512, (ic + 1) * 512)

        def finish():
            # denominator broadcast to all partitions via ones-matmul, then 1/x
            ps_den = ppsum.tile([128, 512], F32, tag="pp", name="ps_den")
            nc.tensor.matmul(
                ps_den[:], lhsT=ones_sb[:], rhs=sums[:], start=True, stop=True
            )

            # PSUM->SBUF attn-out copies gate the next chunk's attnV (av bank
            # reuse): split each copy half DVE / half ACT to halve the stall.
            a_t = []
            for ot in range(CT):
                a = smal.tile([128, 512], F16, tag=f"a{ot}", name=f"a{ot}")
                nc.vector.tensor_copy(out=a[:, 0:256], in_=av[ot][:, 0:256])
                nc.scalar.activation(
                    out=a[:, 256:512], in_=av[ot][:, 256:512], func=Act.Copy
                )
                a_t.append(a)

            # 1/den on a single partition (all 128 rows of ps_den are equal;
            # a full [128,512] DVE reciprocal costs ~4us and stalls the DVE
            # queue that feeds the next chunk), then broadcast it across
            # partitions via a tiny DRAM bounce with a stride-0 partition AP.
            recip_sm = smal.tile([128, 512], F32, tag="recipsm", name="recip_sm", bufs=1)
            nc.vector.reciprocal(out=recip_sm[0:1, :], in_=ps_den[0:1, :])
            recip_d = dramp.tile([1, 512], F32, tag="recipd", name="recip_d")
            nc.sync.dma_start(out=recip_d[:], in_=recip_sm[0:1, :])
            recip = smal.tile([128, 512], F32, tag="recip", name="recip")
            nc.sync.dma_start(out=recip[:], in_=recip_d.to_broadcast([128, 512]))

            for ot2 in range(CT):
                osl = slice(ot2 * 128, (ot2 + 1) * 128)
                ps_o = ppsum.tile([128, 512], F32, tag="pp", name="ps_o")
                for ot in range(CT):
                    nc.tensor.matmul(
                        ps_o[:],
                        lhsT=wo3[:, ot, osl],
                        rhs=a_t[ot][:],
                        start=(ot == 0),
                        stop=(ot == CT - 1),
                    )
                res_t = resp.tile([128, 512], F32, tag="res", name="res_t")
                nc.sync.dma_start(out=res_t[:], in_=d["xr"][osl, isl])
                f1 = finp.tile([128, 512], F32, tag="f1", name="f1")
                nc.vector.tensor_tensor(out=f1[:], in0=ps_o[:], in1=recip[:], op=mult)
                nc.vector.scalar_tensor_tensor(
                    out=f1[:],
                    in0=f1[:],
                    scalar=bo_sb[:, ot2 : ot2 + 1],
                    in1=res_t[:],
                    op0=add,
                    op1=add,
                )
                nc.sync.dma_start(out=d["out"][osl, isl], in_=f1[:])

        return finish

    finish_prev = None
    for ic in range(SH // 512):
        isl = slice(ic * 512, (ic + 1) * 512)
        av = [
            apsum.tile([128, 512], F32, tag=f"av{ot}", name=f"av{ot}")
            for ot in range(CT)
        ]
        sums = smal.tile([128, 512], F32, tag="sums", name="sums")

        def scores_exp(jb):
            ps_s = ppsum.tile([128, 512], F32, tag="ps", name="ps_s")
            for t in range(CT):
                nc.tensor.matmul(
                    ps_s[:],
                    lhsT=kt3[:, t, jb * 128 : (jb + 1) * 128],
                    rhs=qt3[:, t, isl],
                    start=(t == 0),
                    stop=(t == CT - 1),
                )
            e_t = expp.tile([128, 512], F16, tag="exp", name="e_t")
            nc.scalar.activation(
                out=e_t[:], in_=ps_s[:], func=Act.Exp, bias=expb_t[:], scale=float(SCALE)
            )
            return e_t

        def attnv_sums(jb, e_t):
            for ot in range(CT):
                nc.tensor.matmul(
                    av[ot][:],
                    lhsT=v3[:, jb, ot * 128 : (ot + 1) * 128],
                    rhs=e_t[:],
                    start=(jb == 0),
                    stop=(jb == NJB - 1),
                )
            if jb == 0:
                nc.vector.tensor_copy(out=sums[:], in_=e_t[:])
            else:
                nc.vector.tensor_add(out=sums[:], in0=sums[:], in1=e_t[:])

        # Boundary order: two score groups, then the PREVIOUS chunk's whole
        # epilogue (denominator + attn-out copies + out-proj, ~25 PE matmuls),
        # then the first attnV. The attn-out copies that gate this chunk's
        # attnV (av bank reuse) drain while PE runs scores + prev epilogue.
        e0 = scores_exp(0)
        e1 = scores_exp(1)
        if finish_prev is not None:
            finish_prev()
            finish_prev = None
        attnv_sums(0, e0)
        attnv_sums(1, e1)
        for jb in range(2, NJB):
            e_t = scores_exp(jb)
            attnv_sums(jb, e_t)
        finish_prev = make_finisher(ic, av, sums)
    finish_prev()


_CACHE = {}


def _get_program():
    if "nc" in _CACHE:
        return _CACHE["nc"]
    nc = bacc.Bacc("TRN2", target_bir_lowering=False, debug=False, num_devices=N_CORES)
    d = {}
    d["x"] = nc.dram_tensor("x", [C, S], F16, kind="ExternalInput").ap()
    d["xr"] = nc.dram_tensor("xr", [C, SH], F32, kind="ExternalInput").ap()
    for name in ("wqt", "wkt", "wvt", "wot"):
        d[name] = nc.dram_tensor(name, [C, C], F16, kind="ExternalInput").ap()
    for name in ("bq2", "bk2", "bo2", "gw2", "gb2"):
        d[name] = nc.dram_tensor(name, [128, CT], F32, kind="ExternalInput").ap()
    d["bvb"] = nc.dram_tensor("bvb", [128, C], F32, kind="ExternalInput").ap()
    d["gmat"] = nc.dram_tensor("gmat", [128, 128], F32, kind="ExternalInput").ap()
    d["out"] = nc.dram_tensor("out", [C, SH], F32, kind="ExternalOutput").ap()

    with tile.TileContext(nc) as tc:
        with ExitStack() as ctx:
            _build_kernel(ctx, tc, d)
    nc.compile()
    _CACHE["nc"] = nc
    return nc


def make_in_maps(**inputs):
    """Per-core input dicts (numpy). Core c handles batch c//2, query-half c%2."""
    f32 = np.float32
    hs = np.asarray(inputs["hidden_states"], f32).reshape(B, C, S)
    common = {}
    for wname, key in (("wqt", "wq"), ("wkt", "wk"), ("wvt", "wv"), ("wot", "wo")):
        w = np.asarray(inputs[key], f32)
        common[wname] = np.ascontiguousarray(w.T.astype(np.float16))
    for bname, key in (("bq2", "bq"), ("bk2", "bk"), ("bo2", "bo")):
        b = np.asarray(inputs[key], f32)
        common[bname] = np.ascontiguousarray(b.reshape(CT, 128).T)
    common["gw2"] = np.ascontiguousarray(
        np.asarray(inputs["gn_weight"], f32).reshape(CT, 128).T
    )
    common["gb2"] = np.ascontiguousarray(
        np.asarray(inputs["gn_bias"], f32).reshape(CT, 128).T
    )
    common["bvb"] = np.ascontiguousarray(
        np.broadcast_to(np.asarray(inputs["bv"], f32), (128, C))
    )
    gmat = np.zeros((128, 128), f32)
    for g in range(128 // GSIZE):
        gmat[g * GSIZE : (g + 1) * GSIZE, g * GSIZE : (g + 1) * GSIZE] = 1.0 / GSIZE
    common["gmat"] = gmat

    in_maps = []
    for core in range(N_CORES):
        b_idx, half = divmod(core, 2)
        xb = hs[b_idx]
        if half:
            xp = np.concatenate([xb[:, SH:], xb[:, :SH]], axis=1)
        else:
            xp = xb
        m = dict(common)
        m["x"] = np.ascontiguousarray(xp.astype(np.float16))
        m["xr"] = np.ascontiguousarray(xp[:, :SH])
        in_maps.append(m)
    return in_maps


def assemble_output(results):
    out = np.empty((B, C, S), np.float32)
    for core in range(N_CORES):
        b_idx, half = divmod(core, 2)
        out[b_idx][:, half * SH : (half + 1) * SH] = results[core]["out"]
    return out.reshape(B, C, 64, 64)


def run(trace=False, **inputs):
    nc = _get_program()
    in_maps = make_in_maps(**inputs)
    res = run_bass_kernel_spmd(nc, in_maps, core_ids=list(range(N_CORES)), trace=trace)
    return assemble_output(res.results), res


def kernel(**inputs):
    out, _ = run(**inputs)
    return out


# revision 40
# speedup vs baseline: 1.2373x; 1.2373x over previous
"""Trainium2 Bass kernel for an AttentionBlock (GroupNorm -> 1-head attention -> proj -> residual).

Problem: hidden_states (4, 512, 64, 64) fp32; GroupNorm(32 groups) then
single-head attention over S=4096 tokens with head_dim=C=512, output
projection, residual add.

Sharding: 8 cores = 4 batch elements x 2 query-halves. Each core:
 - receives the full [512, 4096] (channels x spatial) slab for its batch
   element, spatially rotated so that *its* 2048 queries are columns 0:2048
   (attention is permutation-invariant over keys, so every core can run the
   identical SPMD program);
 - computes GroupNorm + K/V for all 4096 tokens (redundant x2, cheap) and
   Q only for its half;
 - computes scores^T (keys-on-partition layout), exp, attn @ V, out-proj,
   residual -- no on-chip transposes anywhere.

Numerics: fp16 matmul operands with fp32 PSUM accumulation; softmax without
max-subtraction (scores ~ N(0,1)) but with a constant exp-bias of -4 to keep
unnormalized sums in fp16 range; normalization deferred past the output
projection ((P@V)@Wo / den == (P/den @ V)@Wo). Measured end-to-end absmax
relative error vs fp32 reference: 4.3e-5.

Measured on 8 axon TRN2 cores: ~381us HW exec (~455us when the chip sits in
its throttled P0 clock state); TensorE occupancy ~90%.
"""

from contextlib import ExitStack

import numpy as np

import concourse.bacc as bacc
import concourse.bass as bass
import concourse.tile as tile
from concourse import mybir
from concourse.bass_utils import run_bass_kernel_spmd

F32 = mybir.dt.float32
F16 = mybir.dt.float16

B = 4
C = 512
S = 4096  # 64*64 tokens
SH = S // 2  # tokens per core (query half)
GROUPS = 32
GSIZE = C // GROUPS  # 16 channels per group
EPS = 1e-6
CT = C // 128  # 4 channel tiles
SCALE = 1.0 / np.sqrt(np.float32(C))
EXPBIAS = -4.0  # constant max-substitute inside exp; cancels in normalization

N_CORES = 8


def _build_kernel(ctx: ExitStack, tc: tile.TileContext, d):
    nc = tc.nc
    mult = mybir.AluOpType.mult
    add = mybir.AluOpType.add
    subtract = mybir.AluOpType.subtract
    Act = mybir.ActivationFunctionType

    cst = ctx.enter_context(tc.tile_pool(name="cst", bufs=1))
    xin = ctx.enter_context(tc.tile_pool(name="xin", bufs=3))
    gnp = ctx.enter_context(tc.tile_pool(name="gnp", bufs=4))
    big = ctx.enter_context(tc.tile_pool(name="big", bufs=1))
    expp = ctx.enter_context(tc.tile_pool(name="expp", bufs=4))
    smal = ctx.enter_context(tc.tile_pool(name="smal", bufs=2))
    resp = ctx.enter_context(tc.tile_pool(name="resp", bufs=2))
    dramp = ctx.enter_context(tc.tile_pool(name="dramp", bufs=2, space="DRAM"))
    finp = ctx.enter_context(tc.tile_pool(name="finp", bufs=2))

    x_d = d["x"]  # fp16 copy of the input slab: GN stats + matmul path
    # gmat/gn-affine first (tiny): they gate the GroupNorm matmul + normalize
    gmat_raw = cst.tile([128, 128], F32, tag="gmat_raw")
    nc.sync.dma_start(out=gmat_raw[:], in_=d["gmat"][:])
    gw_sb = cst.tile([128, CT], F32, tag="gw")
    nc.sync.dma_start(out=gw_sb[:], in_=d["gw2"][:])
    gb_sb = cst.tile([128, CT], F32, tag="gb")
    nc.sync.dma_start(out=gb_sb[:], in_=d["gb2"][:])
    # x next: its DMA latency gates GroupNorm and everything after. Two
    # sub-DMAs per channel tile so bn_stats can start on the first half early;
    # each tile gets its own slot so all four transfers issue immediately.
    x_tiles = []
    for t in range(CT):
        x_t = xin.tile([128, S], F16, tag=f"xt{t}", name=f"xt{t}", bufs=1)
        for h in range(4):
            nc.sync.dma_start(
                out=x_t[:, h * (S // 4) : (h + 1) * (S // 4)],
                in_=x_d[t * 128 : (t + 1) * 128, h * (S // 4) : (h + 1) * (S // 4)],
            )
        x_tiles.append(x_t)

    # ---- constants / weights to SBUF (gpsimd DMA queue; overlaps x).
    # Order = first-use order: K/Q/V weights gate the projections (~15us),
    # biases gate the PSUM->SBUF copies a bit later, wo3/bo much later.
    wq3 = cst.tile([128, CT, C], F16, tag="wq3")
    wk3 = cst.tile([128, CT, C], F16, tag="wk3")
    wv3 = cst.tile([128, CT, C], F16, tag="wv3")
    wo3 = cst.tile([128, CT, C], F16, tag="wo3")
    for w_sb, w_d in ((wk3, d["wkt"]), (wq3, d["wqt"]), (wv3, d["wvt"])):
        nc.gpsimd.dma_start(out=w_sb[:], in_=w_d.rearrange("(t p) o -> p t o", p=128))
    bq_sb = cst.tile([128, CT], F32, tag="bq")
    bk_sb = cst.tile([128, CT], F32, tag="bk")
    bo_sb = cst.tile([128, CT], F32, tag="bo")
    for t_sb, t_d in ((bk_sb, d["bk2"]), (bq_sb, d["bq2"]), (bo_sb, d["bo2"])):
        nc.gpsimd.dma_start(out=t_sb[:], in_=t_d[:])
    bvb_sb = cst.tile([128, C], F32, tag="bvb")
    nc.gpsimd.dma_start(out=bvb_sb[:], in_=d["bvb"][:])
    nc.gpsimd.dma_start(out=wo3[:], in_=d["wot"].rearrange("(t p) o -> p t o", p=128))
    # staging copy: the first PE matmul then depends only on the DVE
    # semaphore (S3_LW allows a single wait)
    gmat_sb = cst.tile([128, 128], F32, tag="gmat")
    nc.vector.tensor_copy(out=gmat_sb[:], in_=gmat_raw[:])
    ones_sb = cst.tile([128, 128], F32, tag="ones")
    nc.vector.memset(ones_sb[:], 1.0)
    eps_t = cst.tile([128, 1], F32, tag="epsc")
    nc.vector.memset(eps_t[:], float(EPS))
    expb_t = cst.tile([128, 1], F32, tag="expb")
    nc.vector.memset(expb_t[:], float(EXPBIAS))

    # proj-phase PSUM pool: 6 banks; scoped so its banks are released to the
    # attention pools afterwards
    proj_ctx = ExitStack()
    pjsum = proj_ctx.enter_context(tc.tile_pool(name="pjsum", bufs=6, space="PSUM"))

    # PE warmup: keep TensorE busy during the initial x DMA so HAM reaches
    # K=8/8 before real matmuls; fp32 ones matmuls, one PSUM bank, serial.
    wu = pjsum.tile([128, 128], F32, tag="wu", bufs=1)
    for _ in range(30):
        nc.tensor.matmul(wu[:], lhsT=ones_sb[:], rhs=ones_sb[:], start=True, stop=True)
    # ---- GroupNorm: per channel-tile stats -> group stats -> normalize ----
    xg3 = big.tile([128, CT, S], F16, tag="xg3")  # normalized input, [c, s]
    for t in range(CT):
        x_t = x_tiles[t]
        stats = gnp.tile([128, 8, 6], F32, tag="stats")
        xr = x_t.rearrange("p (n f) -> p n f", f=512)
        for i in range(8):
            nc.vector.bn_stats(out=stats[:, i, :], in_=xr[:, i, :])
        mv = gnp.tile([128, 2], F32, tag="mv")
        nc.vector.bn_aggr(out=mv[:], in_=stats[:])

        # group-average mean/var and mean^2 across the 16 partitions of each
        # group via PE matmuls; each rhs has a single producer instruction
        # (matmul LDWEIGHTS lowering only supports a couple of sem waits).
        mv3 = gnp.tile([128, 3], F32, tag="mv3")
        nc.vector.tensor_copy(out=mv3[:, 0:2], in_=mv[:])
        nc.vector.tensor_tensor(out=mv3[:, 2:3], in0=mv[:, 0:1], in1=mv[:, 0:1], op=mult)
        ps_g = pjsum.tile([128, 3], F32, tag="pj")
        nc.tensor.matmul(ps_g[:], lhsT=gmat_sb[:], rhs=mv3[:], start=True, stop=True)

        # gstat = [mean_g, avg var_p, avg mean_p^2]
        gstat = gnp.tile([128, 3], F32, tag="gstat")
        nc.vector.tensor_copy(out=gstat[:], in_=ps_g[:])
        # var_g = avg(var_p) + avg(mean_p^2) - mean_g^2
        varg = gnp.tile([128, 1], F32, tag="varg")
        nc.vector.tensor_tensor(out=varg[:], in0=gstat[:, 0:1], in1=gstat[:, 0:1], op=mult)
        nc.vector.tensor_tensor(out=varg[:], in0=gstat[:, 2:3], in1=varg[:], op=subtract)
        nc.vector.tensor_tensor(out=varg[:], in0=gstat[:, 1:2], in1=varg[:], op=add)
        stdt = gnp.tile([128, 1], F32, tag="stdt")
        nc.scalar.activation(out=stdt[:], in_=varg[:], func=Act.Sqrt, bias=eps_t[:])
        rstd = gnp.tile([128, 1], F32, tag="rstd")
        nc.vector.reciprocal(out=rstd[:], in_=stdt[:])

        scl = gnp.tile([128, 1], F32, tag="scl")
        nc.vector.tensor_tensor(out=scl[:], in0=rstd[:], in1=gw_sb[:, t : t + 1], op=mult)
        sft = gnp.tile([128, 1], F32, tag="sft")
        nc.vector.tensor_tensor(out=sft[:], in0=gstat[:, 0:1], in1=scl[:], op=mult)
        nc.vector.tensor_tensor(out=sft[:], in0=gb_sb[:, t : t + 1], in1=sft[:], op=subtract)

        # alternate the normalize between ACT and DVE so consecutive channel
        # tiles normalize in parallel on the two engines
        if t % 2 == 0:
            nc.scalar.activation(
                out=xg3[:, t, :],
                in_=x_t[:],
                func=Act.Identity,
                bias=sft[:],
                scale=scl[:],
            )
        else:
            nc.vector.tensor_scalar(
                out=xg3[:, t, :],
                in0=x_t[:],
                scalar1=scl[:],
                scalar2=sft[:],
                op0=mult,
                op1=add,
            )

    # ---- projections ----
    kt3 = big.tile([128, CT, S], F16, tag="kt3")  # k^T [c, j]
    for ot in range(CT):
        for jc in range(S // 512):
            ps = pjsum.tile([128, 512], F32, tag="pj")
            for t in range(CT):
                nc.tensor.matmul(
                    ps[:],
                    lhsT=wk3[:, t, ot * 128 : (ot + 1) * 128],
                    rhs=xg3[:, t, jc * 512 : (jc + 1) * 512],
                    start=(t == 0),
                    stop=(t == CT - 1),
                )
            nc.scalar.activation(
                out=kt3[:, ot, jc * 512 : (jc + 1) * 512],
                in_=ps[:],
                func=Act.Identity,
                bias=bk_sb[:, ot : ot + 1],
            )

    qt3 = big.tile([128, CT, SH], F16, tag="qt3")  # q^T [c, i]
    for ot in range(CT):
        for ic in range(SH // 512):
            ps = pjsum.tile([128, 512], F32, tag="pj")
            for t in range(CT):
                nc.tensor.matmul(
                    ps[:],
                    lhsT=wq3[:, t, ot * 128 : (ot + 1) * 128],
                    rhs=xg3[:, t, ic * 512 : (ic + 1) * 512],
                    start=(t == 0),
                    stop=(t == CT - 1),
                )
            nc.scalar.activation(
                out=qt3[:, ot, ic * 512 : (ic + 1) * 512],
                in_=ps[:],
                func=Act.Identity,
                bias=bq_sb[:, ot : ot + 1],
            )

    v3 = big.tile([128, S // 128, C], F16, tag="v3")  # v natural [j, o]
    for jb in range(S // 128):
        ps = pjsum.tile([128, 512], F32, tag="pj")
        for t in range(CT):
            nc.tensor.matmul(
                ps[:],
                lhsT=xg3[:, t, jb * 128 : (jb + 1) * 128],
                rhs=wv3[:, t, :],
                start=(t == 0),
                stop=(t == CT - 1),
            )
        nc.vector.tensor_add(out=v3[:, jb, :], in0=ps[:], in1=bvb_sb[:])

    # release the 6 proj banks, then open the attention PSUM pools:
    # ps(2) + av0..3(1 each) + pp(2) = 8 banks
    proj_ctx.close()
    ppsum = ctx.enter_context(tc.tile_pool(name="ppsum", bufs=2, space="PSUM"))
    apsum = ctx.enter_context(tc.tile_pool(name="apsum", bufs=1, space="PSUM"))

    # ---- attention + output projection, per 512-query chunk ----
    # The per-chunk epilogue (denominator, attn-out copies, output projection,
    # residual) is deferred into the next chunk's j-loop so its PE work and
    # PSUM->SBUF copies overlap the next chunk's score matmuls.
    NJB = S // 128  # 32 key blocks

    def make_finisher(ic, av, sums):
        isl = slice(ic * 512, (ic + 1) * 512)
        state = {}

        def finish_a():
            # denominator broadcast to all partitions via ones-matmul
            ps_den = ppsum.tile([128, 512], F32, tag="pp", name="ps_den")
            nc.tensor.matmul(
                ps_den[:], lhsT=ones_sb[:], rhs=sums[:], start=True, stop=True
            )
            # PSUM->SBUF attn-out copies gate the next chunk's attnV (av bank
            # reuse): split each copy half DVE / half ACT to halve the stall.
            a_t = []
            for ot in range(CT):
                a = smal.tile([128, 512], F16, tag=f"a{ot}", name=f"a{ot}")
                nc.vector.tensor_copy(out=a[:, 0:256], in_=av[ot][:, 0:256])
                nc.scalar.activation(
                    out=a[:, 256:512], in_=av[ot][:, 256:512], func=Act.Copy
                )
                a_t.append(a)
            state["ps_den"] = ps_den
            state["a_t"] = a_t

        def finish_b():
            ps_den, a_t = state["ps_den"], state["a_t"]
            recip = smal.tile([128, 512], F32, tag="recip", name="recip")
            nc.vector.reciprocal(out=recip[:], in_=ps_den[:])

            for ot2 in range(CT):
                osl = slice(ot2 * 128, (ot2 + 1) * 128)
                ps_o = ppsum.tile([128, 512], F32, tag="pp", name="ps_o")
                for ot in range(CT):
                    nc.tensor.matmul(
                        ps_o[:],
                        lhsT=wo3[:, ot, osl],
                        rhs=a_t[ot][:],
                        start=(ot == 0),
                        stop=(ot == CT - 1),
                    )
                res_t = resp.tile([128, 512], F32, tag="res", name="res_t")
                nc.sync.dma_start(out=res_t[:], in_=d["xr"][osl, isl])
                f1 = finp.tile([128, 512], F32, tag="f1", name="f1")
                nc.vector.tensor_tensor(out=f1[:], in0=ps_o[:], in1=recip[:], op=mult)
                nc.vector.scalar_tensor_tensor(
                    out=f1[:],
                    in0=f1[:],
                    scalar=bo_sb[:, ot2 : ot2 + 1],
                    in1=res_t[:],
                    op0=add,
                    op1=add,
                )
                nc.sync.dma_start(out=d["out"][osl, isl], in_=f1[:])

        return finish_a, finish_b

    finish_prev = None
    for ic in range(SH // 512):
        isl = slice(ic * 512, (ic + 1) * 512)
        av = [
            apsum.tile([128, 512], F32, tag=f"av{ot}", name=f"av{ot}")
            for ot in range(CT)
        ]
        sums = smal.tile([128, 512], F32, tag="sums", name="sums")

        def scores_exp(jb):
            ps_s = ppsum.tile([128, 512], F32, tag="ps", name="ps_s")
            for t in range(CT):
                nc.tensor.matmul(
                    ps_s[:],
                    lhsT=kt3[:, t, jb * 128 : (jb + 1) * 128],
                    rhs=qt3[:, t, isl],
                    start=(t == 0),
                    stop=(t == CT - 1),
                )
            e_t = expp.tile([128, 512], F16, tag="exp", name="e_t")
            nc.scalar.activation(
                out=e_t[:], in_=ps_s[:], func=Act.Exp, bias=expb_t[:], scale=float(SCALE)
            )
            return e_t

        def attnv_sums(jb, e_t):
            for ot in range(CT):
                nc.tensor.matmul(
                    av[ot][:],
                    lhsT=v3[:, jb, ot * 128 : (ot + 1) * 128],
                    rhs=e_t[:],
                    start=(jb == 0),
                    stop=(jb == NJB - 1),
                )
            if jb == 0:
                nc.vector.tensor_copy(out=sums[:], in_=e_t[:])
            else:
                nc.vector.tensor_add(out=sums[:], in0=sums[:], in1=e_t[:])

        # Boundary order: two score groups first, then the previous chunk's
        # epilogue part A (denominator + the av-bank-releasing copies), then
        # this chunk's first attnV. Epilogue part B (4us DVE reciprocal +
        # out-proj + residual fuse) is emitted after several j-iterations so
        # its DVE work queues behind this chunk's sums ops (which release
        # exp-pool slots the score pipeline needs).
        e0 = scores_exp(0)
        e1 = scores_exp(1)
        if finish_prev is not None:
            finish_prev[0]()
        attnv_sums(0, e0)
        attnv_sums(1, e1)
        for jb in range(2, NJB):
            e_t = scores_exp(jb)
            attnv_sums(jb, e_t)
            if jb == 6 and finish_prev is not None:
                finish_prev[1]()
                finish_prev = None
        finish_prev = make_finisher(ic, av, sums)
    finish_prev[0]()
    finish_prev[1]()


_CACHE = {}


def _get_program():
    if "nc" in _CACHE:
        return _CACHE["nc"]
    nc = bacc.Bacc("TRN2", target_bir_lowering=False, debug=False, num_devices=N_CORES)
    d = {}
    d["x"] = nc.dram_tensor("x", [C, S], F16, kind="ExternalInput").ap()
    d["xr"] = nc.dram_tensor("xr", [C, SH], F32, kind="ExternalInput").ap()
    for name in ("wqt", "wkt", "wvt", "wot"):
        d[name] = nc.dram_tensor(name, [C, C], F16, kind="ExternalInput").ap()
    for name in ("bq2", "bk2", "bo2", "gw2", "gb2"):
        d[name] = nc.dram_tensor(name, [128, CT], F32, kind="ExternalInput").ap()
    d["bvb"] = nc.dram_tensor("bvb", [128, C], F32, kind="ExternalInput").ap()
    d["gmat"] = nc.dram_tensor("gmat", [128, 128], F32, kind="ExternalInput").ap()
    d["out"] = nc.dram_tensor("out", [C, SH], F32, kind="ExternalOutput").ap()

    with tile.TileContext(nc) as tc:
        with ExitStack() as ctx:
            _build_kernel(ctx, tc, d)
    nc.compile()
    _CACHE["nc"] = nc
    return nc


def make_in_maps(**inputs):
    """Per-core input dicts (numpy). Core c handles batch c//2, query-half c%2."""
    f32 = np.float32
    hs = np.asarray(inputs["hidden_states"], f32).reshape(B, C, S)
    common = {}
    for wname, key in (("wqt", "wq"), ("wkt", "wk"), ("wvt", "wv"), ("wot", "wo")):
        w = np.asarray(inputs[key], f32)
        common[wname] = np.ascontiguousarray(w.T.astype(np.float16))
    for bname, key in (("bq2", "bq"), ("bk2", "bk"), ("bo2", "bo")):
        b = np.asarray(inputs[key], f32)
        common[bname] = np.ascontiguousarray(b.reshape(CT, 128).T)
    common["gw2"] = np.ascontiguousarray(
        np.asarray(inputs["gn_weight"], f32).reshape(CT, 128).T
    )
    common["gb2"] = np.ascontiguousarray(
        np.asarray(inputs["gn_bias"], f32).reshape(CT, 128).T
    )
    common["bvb"] = np.ascontiguousarray(
        np.broadcast_to(np.asarray(inputs["bv"], f32), (128, C))
    )
    gmat = np.zeros((128, 128), f32)
    for g in range(128 // GSIZE):
        gmat[g * GSIZE : (g + 1) * GSIZE, g * GSIZE : (g + 1) * GSIZE] = 1.0 / GSIZE
    common["gmat"] = gmat

    in_maps = []
    for core in range(N_CORES):
        b_idx, half = divmod(core, 2)
        xb = hs[b_idx]
        if half:
            xp = np.concatenate([xb[:, SH:], xb[:, :SH]], axis=1)
        else:
            xp = xb
        m = dict(common)
        m["x"] = np.ascontiguousarray(xp.astype(np.float16))
        m["xr"] = np.ascontiguousarray(xp[:, :SH])
        in_maps.append(m)
    return in_maps


def assemble_output(results):
    out = np.empty((B, C, S), np.float32)
    for core in range(N_CORES):
        b_idx, half = divmod(core, 2)
        out[b_idx][:, half * SH : (half + 1) * SH] = results[core]["out"]
    return out.reshape(B, C, 64, 64)


def run(trace=False, **inputs):
    nc = _get_program()
    in_maps = make_in_maps(**inputs)
    res = run_bass_kernel_spmd(nc, in_maps, core_ids=list(range(N_CORES)), trace=trace)
    return assemble_output(res.results), res


def kernel(**inputs):
    out, _ = run(**inputs)
    return out
